# revision 2
# baseline (speedup 1.0000x reference)
"""Trainium2 Bass kernel for nn_DyConv (MoE routed dynamic conv).

Model (per batch image b):
  g = mean(x[b], spatial)                      # [C]
  w = softmax(fc2(relu(fc1(g))))               # [E]  router weights
  out[b] = sum_e w[e] * silu(bn_e(conv3x3_e(x[b])))

Strategy: pure data-parallel over batch. B=16 images / 8 cores = 2 images
per core; router + experts replicated. No collectives.

Per-core device program (per image):
  - x arrives host-padded to a flat 162x162 zero-padded layout in bf16.
  - Two SBUF "region" copies per half-image stack 2 shifted taps on the
    128 partitions: A = [x(+0); x(+1)], B = [x(+2); x(+164)].  A K=128
    matmul against A at offset o contracts taps (o, o+1) for all 64
    channels; 9 conv taps = 5 matmuls (3xA-pair, 1xB-pair, 1 half-K
    single) per expert-pair.  Two expert pairs stacked in M=128.
  - BN scale folded into conv weights on host; BN shift applied as the
    per-partition bias of the SiLU activation (ScalarE, PSUM->SBUF bf16).
  - Router: VectorE reduces over the bf16 image; tiny matmuls + a
    tanh-based exp for the 4-way softmax (keeps everything in the
    silu_and_others act table set); the mixing weights become two
    scaled-identity bf16 [128,64] lhsT tiles built on ScalarE.
  - Mix: 2 accumulating matmuls -> PSUM [64,N]; VectorE copy to SBUF;
    strided DMA (drops the 2 pad columns) to the output.
"""
import os
import sys
import numpy as np

if "/opt/trn_rl_repo" not in sys.path:
    sys.path.insert(0, "/opt/trn_rl_repo")

import ml_dtypes  # noqa: E402

BF16_NP = ml_dtypes.bfloat16

B, C, H, W = 16, 64, 160, 160
E, R = 4, 16
NCORES = 8
IMG_PER_CORE = B // NCORES          # 2
WP = W + 2                          # 162 padded row
LP = (H + 2) * WP                   # 26244 padded flat image
LHOST = 26600                       # host buffer with zero margin (max read 26408)
RLEN = 82 * WP                      # 13284: half-image region (80 out rows + 2 halo)
HB = 80 * WP                        # 12960: out-grid columns per half
NT = 486                            # psum tile = 3 out rows
BN_EPS = 1e-3

_CACHE = {}


def _build_program(reps=1):
    import concourse.bacc as bacc
    import concourse.tile as tile
    from concourse import mybir

    BF16 = mybir.dt.bfloat16
    F32 = mybir.dt.float32
    AF = mybir.ActivationFunctionType
    ALU = mybir.AluOpType
    AX = mybir.AxisListType

    nc = bacc.Bacc("TRN2", target_bir_lowering=False, debug=False,
                   num_devices=NCORES)

    xp_d = nc.dram_tensor("xp", [IMG_PER_CORE, C, LHOST], BF16, kind="ExternalInput")
    wk_d = nc.dram_tensor("wk", [128, 1024], BF16, kind="ExternalInput")
    wks_d = nc.dram_tensor("wks", [64, 256], BF16, kind="ExternalInput")
    fc1t_d = nc.dram_tensor("fc1t", [64, 16], BF16, kind="ExternalInput")
    fc2t_d = nc.dram_tensor("fc2t", [16, 4], BF16, kind="ExternalInput")
    fc2b_d = nc.dram_tensor("fc2b", [4, 1], F32, kind="ExternalInput")
    bnb_d = nc.dram_tensor("bnb", [128, 2], F32, kind="ExternalInput")
    idc_d = nc.dram_tensor("idc", [128, 64], F32, kind="ExternalInput")
    o64_d = nc.dram_tensor("o64", [1, 64], F32, kind="ExternalInput")
    out_d = nc.dram_tensor("out", [IMG_PER_CORE, C, H, W], F32, kind="ExternalOutput")

    with tile.TileContext(nc) as tc:
        with tc.tile_pool(name="consts", bufs=1) as cp, \
             tc.tile_pool(name="regs", bufs=2) as rp, \
             tc.tile_pool(name="work", bufs=2) as wp, \
             tc.tile_pool(name="bounce", bufs=8) as bp, \
             tc.tile_pool(name="psum", bufs=1, space="PSUM") as pp:

            wk_sb = cp.tile([128, 1024], BF16)
            nc.sync.dma_start(wk_sb[:, :], wk_d[:, :])
            wks_sb = cp.tile([64, 256], BF16)
            nc.sync.dma_start(wks_sb[:, :], wks_d[:, :])
            fc1t_sb = cp.tile([64, 16], BF16)
            nc.sync.dma_start(fc1t_sb[:, :], fc1t_d[:, :])
            fc2t_sb = cp.tile([16, 4], BF16)
            nc.sync.dma_start(fc2t_sb[:, :], fc2t_d[:, :])
            fc2b_sb = cp.tile([4, 1], F32)
            nc.sync.dma_start(fc2b_sb[:, :], fc2b_d[:, :])
            bnb_sb = cp.tile([128, 2], F32)
            nc.sync.dma_start(bnb_sb[:, :], bnb_d[:, :])
            idc_sb = cp.tile([128, 64], F32)
            nc.sync.dma_start(idc_sb[:, :], idc_d[:, :])
            o64_sb = cp.tile([1, 64], F32)
            nc.sync.dma_start(o64_sb[:, :], o64_d[:, :])

            for img in [i % IMG_PER_CORE for i in range(IMG_PER_CORE * reps)]:
                # ---- region loads (2 halves x {A, B}) + partial reduces ----
                regs = []
                parts = wp.tile([64, 2], F32, tag="parts")
                for h in range(2):
                    bh = h * HB
                    regA = rp.tile([128, RLEN], BF16, tag="regA")
                    nc.sync.dma_start(regA[0:64, :], xp_d[img, :, bh:bh + RLEN])
                    nc.sync.dma_start(regA[64:128, :], xp_d[img, :, bh + 1:bh + 1 + RLEN])
                    regB = rp.tile([128, RLEN], BF16, tag="regB")
                    nc.sync.dma_start(regB[0:64, :], xp_d[img, :, bh + 2:bh + 2 + RLEN])
                    nc.sync.dma_start(regB[64:128, :], xp_d[img, :, bh + 164:bh + 164 + RLEN])
                    regs.append((regA, regB))
                    rng = HB if h == 0 else RLEN
                    nc.vector.tensor_reduce(parts[:, h:h + 1], regA[0:64, 0:rng],
                                            axis=AX.X, op=ALU.add)

                # ---- router ----
                gbf = wp.tile([64, 1], BF16, tag="gbf")
                nc.vector.tensor_tensor(gbf[:, :], parts[:, 0:1], parts[:, 1:2], op=ALU.add)
                h_ps = pp.tile([16, 1], F32, tag="pr")
                nc.tensor.matmul(h_ps[:, :], fc1t_sb[:, :], gbf[:, :], start=True, stop=True)
                hbf = wp.tile([16, 1], BF16, tag="hbf")
                nc.scalar.activation(hbf[:, :], h_ps[:, :], AF.Relu)
                l_ps = pp.tile([4, 1], F32, tag="pr")
                nc.tensor.matmul(l_ps[:, :], fc2t_sb[:, :], hbf[:, :], start=True, stop=True)
                lsb = wp.tile([4, 1], F32, tag="lsb")
                nc.scalar.activation(lsb[:, :], l_ps[:, :], AF.Identity, bias=fc2b_sb[:, 0:1])
                lrow_ps = pp.tile([1, 4], F32, tag="pr")
                nc.tensor.transpose(lrow_ps[:, :], lsb[:, :], idc_sb[0:4, 0:4])
                # exp(l) = (1 + tanh(l/2)) / (1 - tanh(l/2)); logits are O(0.5)
                trow = wp.tile([1, 4], F32, tag="trow")
                nc.scalar.activation(trow[:, :], lrow_ps[:, :], AF.Tanh, scale=0.5)
                num = wp.tile([1, 4], F32, tag="num")
                nc.vector.tensor_scalar_add(num[:, :], trow[:, :], 1.0)
                den = wp.tile([1, 4], F32, tag="den")
                nc.vector.tensor_scalar(den[:, :], trow[:, :], -1.0, 1.0,
                                        op0=ALU.mult, op1=ALU.add)
                rec = wp.tile([1, 4], F32, tag="rec")
                nc.vector.reciprocal(rec[:, :], den[:, :])
                erow = wp.tile([1, 4], F32, tag="erow")
                nc.vector.tensor_tensor(erow[:, :], num[:, :], rec[:, :], op=ALU.mult)
                ssum = wp.tile([1, 1], F32, tag="ssum")
                nc.vector.tensor_reduce(ssum[:, :], erow[:, :], axis=AX.X, op=ALU.add)
                sinv = wp.tile([1, 1], F32, tag="sinv")
                nc.vector.reciprocal(sinv[:, :], ssum[:, :])
                wrow = wp.tile([1, 4], F32, tag="wrow")
                nc.vector.tensor_scalar_mul(wrow[:, :], erow[:, :], sinv[:, 0:1])
                rowA = wp.tile([1, 128], F32, tag="rowA")
                nc.vector.tensor_scalar_mul(rowA[:, 0:64], o64_sb[:, :], wrow[:, 0:1])
                nc.vector.tensor_scalar_mul(rowA[:, 64:128], o64_sb[:, :], wrow[:, 1:2])
                rowB = wp.tile([1, 128], F32, tag="rowB")
                nc.vector.tensor_scalar_mul(rowB[:, 0:64], o64_sb[:, :], wrow[:, 2:3])
                nc.vector.tensor_scalar_mul(rowB[:, 64:128], o64_sb[:, :], wrow[:, 3:4])
                wc_ps = pp.tile([128, 2], F32, tag="pw")
                nc.tensor.matmul(wc_ps[:, 0:1], rowA[:, :], idc_sb[0:1, 0:1],
                                 start=True, stop=True)
                nc.tensor.matmul(wc_ps[:, 1:2], rowB[:, :], idc_sb[0:1, 0:1],
                                 start=True, stop=True)
                wcol = wp.tile([128, 2], F32, tag="wcol")
                nc.scalar.copy(wcol[:, :], wc_ps[:, :])
                mixA = wp.tile([128, 64], BF16, tag="mixA")
                nc.scalar.activation(mixA[:, :], idc_sb[:, :], AF.Copy, scale=wcol[:, 0:1])
                mixB = wp.tile([128, 64], BF16, tag="mixB")
                nc.scalar.activation(mixB[:, :], idc_sb[:, :], AF.Copy, scale=wcol[:, 1:2])

                # ---- conv + silu + mix tiles ----
                for h in range(2):
                    regA, regB = regs[h]
                    for t in range(27):
                        N = NT if t < 26 else 324
                        n0 = t * NT
                        tsbs = []
                        for ep in range(2):
                            cps = pp.tile([128, N], F32, tag=f"pc{ep}", bufs=2)
                            base = ep * 512
                            nc.tensor.matmul(cps[:, :], wk_sb[:, base:base + 128],
                                             regA[:, n0:n0 + N], start=True, stop=False)
                            nc.tensor.matmul(cps[:, :], wk_sb[:, base + 128:base + 256],
                                             regA[:, n0 + 162:n0 + 162 + N],
                                             start=False, stop=False)
                            nc.tensor.matmul(cps[:, :], wk_sb[:, base + 256:base + 384],
                                             regA[:, n0 + 324:n0 + 324 + N],
                                             start=False, stop=False)
                            nc.tensor.matmul(cps[:, :], wk_sb[:, base + 384:base + 512],
                                             regB[:, n0:n0 + N], start=False, stop=False)
                            nc.tensor.matmul(cps[:, :], wks_sb[:, ep * 128:ep * 128 + 128],
                                             regB[0:64, n0 + 324:n0 + 324 + N],
                                             start=False, stop=True)
                            tsb = wp.tile([128, N], BF16, tag=f"t{ep}", bufs=12)
                            nc.scalar.activation(tsb[:, :], cps[:, :], AF.Silu,
                                                 bias=bnb_sb[:, ep:ep + 1])
                            tsbs.append(tsb)
                        po = pp.tile([64, N], F32, tag="po", bufs=2)
                        nc.tensor.matmul(po[:, :], mixA[:, :], tsbs[0][:, :],
                                         start=True, stop=False)
                        nc.tensor.matmul(po[:, :], mixB[:, :], tsbs[1][:, :],
                                         start=False, stop=True)
                        bounce = bp.tile([64, N], F32, tag="bounce")
                        nc.vector.tensor_copy(bounce[:, :], po[:, :])
                        r0 = h * 80 + t * 3
                        nrows = 3 if t < 26 else 2
                        bv = bounce[:, :].rearrange("p (r c) -> p r c", c=WP)[:, :, 0:W]
                        nc.sync.dma_start(out_d[img, :, r0:r0 + nrows, :], bv)

    nc.compile()
    return nc


def _prep_weights(fc1_w, fc2_w, fc2_b, conv_w, bn_gamma, bn_beta, bn_mean, bn_var):
    scale = bn_gamma / np.sqrt(bn_var + BN_EPS)            # [E, C]
    shift = bn_beta - bn_mean * scale                      # [E, C]
    ws = conv_w * scale[:, :, None, None, None]            # [E, Co, Ci, 3, 3]

    # paired-tap lhsT blocks: [K=128 (2 taps x 64 ci), M=128 (2 experts x 64 co)]
    groups = [((0, 0), (0, 1)), ((1, 0), (1, 1)), ((2, 0), (2, 1)), ((0, 2), (1, 2))]
    wk = np.zeros((128, 1024), np.float32)
    for ep in range(2):
        for g, (ta, tb) in enumerate(groups):
            blk = np.stack([ws[:, :, :, ta[0], ta[1]], ws[:, :, :, tb[0], tb[1]]])
            blk = blk[:, 2 * ep:2 * ep + 2]                # [j, le, Co, Ci]
            lhsT = blk.transpose(0, 3, 1, 2).reshape(128, 128)
            wk[:, (ep * 4 + g) * 128:(ep * 4 + g + 1) * 128] = lhsT
    wks = np.zeros((64, 256), np.float32)
    s22 = ws[:, :, :, 2, 2]                                # [E, Co, Ci]
    for ep in range(2):
        blk = s22[2 * ep:2 * ep + 2]                       # [le, Co, Ci]
        wks[:, ep * 128:(ep + 1) * 128] = blk.transpose(2, 0, 1).reshape(64, 128)

    return {
        "wk": wk.astype(BF16_NP),
        "wks": wks.astype(BF16_NP),
        "fc1t": (fc1_w.T / float(H * W)).astype(BF16_NP),  # [64, 16]
        "fc2t": fc2_w.T.astype(BF16_NP),                   # [16, 4]
        "fc2b": fc2_b.reshape(4, 1).astype(np.float32),
        "bnb": np.stack([np.concatenate([shift[0], shift[1]]),
                         np.concatenate([shift[2], shift[3]])], axis=1).astype(np.float32),
        "idc": (np.arange(128)[:, None] % 64 == np.arange(64)[None, :]).astype(np.float32),
        "o64": np.ones((1, 64), np.float32),
    }


def kernel(x, fc1_w, fc2_w, fc2_b, conv_w, bn_gamma, bn_beta, bn_mean, bn_var):
    from concourse.bass_utils import run_bass_kernel_spmd

    x = np.asarray(x, np.float32)
    reps = int(os.environ.get("BASS_KERNEL_REPS", "1"))
    key = f"nc{reps}"
    if key not in _CACHE:
        _CACHE[key] = _build_program(reps)
    nc = _CACHE[key]

    wmap = _prep_weights(np.asarray(fc1_w, np.float32), np.asarray(fc2_w, np.float32),
                         np.asarray(fc2_b, np.float32), np.asarray(conv_w, np.float32),
                         np.asarray(bn_gamma, np.float32), np.asarray(bn_beta, np.float32),
                         np.asarray(bn_mean, np.float32), np.asarray(bn_var, np.float32))

    # host-side zero-pad + bf16 cast into the flat 162x162 (+margin) layout
    xp = np.zeros((B, C, LHOST), BF16_NP)
    xpad = xp[:, :, :LP].reshape(B, C, H + 2, WP)
    xpad[:, :, 1:H + 1, 1:W + 1] = x.astype(BF16_NP)

    in_maps = []
    for c in range(NCORES):
        m = dict(wmap)
        m["xp"] = xp[c * IMG_PER_CORE:(c + 1) * IMG_PER_CORE]
        in_maps.append(m)

    trace = bool(int(os.environ.get("BASS_KERNEL_TRACE", "0")))
    res = run_bass_kernel_spmd(nc, in_maps, list(range(NCORES)), trace=trace)
    _CACHE["last_results"] = res
    return np.concatenate([res.results[c]["out"] for c in range(NCORES)], axis=0)


# revision 3
# speedup vs baseline: 1.1274x; 1.1274x over previous
"""Trainium2 Bass kernel for nn_DyConv (MoE routed dynamic conv).

Model (per batch image b):
  g = mean(x[b], spatial)                      # [C]
  w = softmax(fc2(relu(fc1(g))))               # [E]  router weights
  out[b] = sum_e w[e] * silu(bn_e(conv3x3_e(x[b])))

Strategy: pure data-parallel over batch. B=16 images / 8 cores = 2 images
per core; router + experts replicated. No collectives.

Per-core device program (per image):
  - x arrives host-padded to a flat 162x162 zero-padded layout in bf16.
  - Two SBUF "region" copies per half-image stack 2 shifted taps on the
    128 partitions: A = [x(+0); x(+1)], B = [x(+2); x(+164)].  A K=128
    matmul against A at offset o contracts taps (o, o+1) for all 64
    channels; 9 conv taps = 5 matmuls (3xA-pair, 1xB-pair, 1 half-K
    single) per expert-pair.  Two expert pairs stacked in M=128.
  - BN scale folded into conv weights on host; BN shift applied as the
    per-partition bias of the SiLU activation (ScalarE, PSUM->SBUF bf16).
  - Router: VectorE reduces over the bf16 image; tiny matmuls + a
    tanh-based exp for the 4-way softmax (keeps everything in the
    silu_and_others act table set); the mixing weights become two
    scaled-identity bf16 [128,64] lhsT tiles built on ScalarE.
  - Mix: 2 accumulating matmuls -> PSUM [64,N]; VectorE copy to SBUF;
    strided DMA (drops the 2 pad columns) to the output.
"""
import os
import sys
import numpy as np

if "/opt/trn_rl_repo" not in sys.path:
    sys.path.insert(0, "/opt/trn_rl_repo")

import ml_dtypes  # noqa: E402

BF16_NP = ml_dtypes.bfloat16

B, C, H, W = 16, 64, 160, 160
E, R = 4, 16
NCORES = 8
IMG_PER_CORE = B // NCORES          # 2
WP = W + 2                          # 162 padded row
LP = (H + 2) * WP                   # 26244 padded flat image
LHOST = 26600                       # host buffer with zero margin (max read 26408)
RLEN = 82 * WP                      # 13284: half-image region (80 out rows + 2 halo)
HB = 80 * WP                        # 12960: out-grid columns per half
NT = 486                            # psum tile = 3 out rows
BN_EPS = 1e-3

_CACHE = {}


def _build_program(reps=1):
    import concourse.bacc as bacc
    import concourse.tile as tile
    from concourse import mybir

    BF16 = mybir.dt.bfloat16
    F32 = mybir.dt.float32
    AF = mybir.ActivationFunctionType
    ALU = mybir.AluOpType
    AX = mybir.AxisListType

    nc = bacc.Bacc("TRN2", target_bir_lowering=False, debug=False,
                   num_devices=NCORES)

    xp_d = nc.dram_tensor("xp", [IMG_PER_CORE, C, LHOST], BF16, kind="ExternalInput")
    wk_d = nc.dram_tensor("wk", [128, 1024], BF16, kind="ExternalInput")
    wks_d = nc.dram_tensor("wks", [128, 256], BF16, kind="ExternalInput")
    fc1t_d = nc.dram_tensor("fc1t", [64, 16], BF16, kind="ExternalInput")
    fc2t_d = nc.dram_tensor("fc2t", [16, 4], BF16, kind="ExternalInput")
    fc2b_d = nc.dram_tensor("fc2b", [4, 1], F32, kind="ExternalInput")
    bnb_d = nc.dram_tensor("bnb", [128, 2], F32, kind="ExternalInput")
    idc_d = nc.dram_tensor("idc", [128, 128], F32, kind="ExternalInput")
    o64_d = nc.dram_tensor("o64", [1, 64], F32, kind="ExternalInput")
    out_d = nc.dram_tensor("out", [IMG_PER_CORE, C, H, W], F32, kind="ExternalOutput")

    with tile.TileContext(nc) as tc:
        with tc.tile_pool(name="consts", bufs=1) as cp, \
             tc.tile_pool(name="regs", bufs=2) as rp, \
             tc.tile_pool(name="work", bufs=2) as wp, \
             tc.tile_pool(name="bounce", bufs=8) as bp, \
             tc.tile_pool(name="psum", bufs=1, space="PSUM") as pp:

            wk_sb = cp.tile([128, 1024], BF16)
            nc.sync.dma_start(wk_sb[:, :], wk_d[:, :])
            wks_sb = cp.tile([128, 256], BF16)
            nc.sync.dma_start(wks_sb[:, :], wks_d[:, :])
            fc1t_sb = cp.tile([64, 16], BF16)
            nc.sync.dma_start(fc1t_sb[:, :], fc1t_d[:, :])
            fc2t_sb = cp.tile([16, 4], BF16)
            nc.sync.dma_start(fc2t_sb[:, :], fc2t_d[:, :])
            fc2b_sb = cp.tile([4, 1], F32)
            nc.sync.dma_start(fc2b_sb[:, :], fc2b_d[:, :])
            bnb_sb = cp.tile([128, 2], F32)
            nc.sync.dma_start(bnb_sb[:, :], bnb_d[:, :])
            idc_sb = cp.tile([128, 128], F32)
            nc.sync.dma_start(idc_sb[:, :], idc_d[:, :])
            o64_sb = cp.tile([1, 64], F32)
            nc.sync.dma_start(o64_sb[:, :], o64_d[:, :])

            for img in [i % IMG_PER_CORE for i in range(IMG_PER_CORE * reps)]:
                # ---- region loads (2 halves x {A, B}) + partial reduces ----
                regs = []
                parts = wp.tile([64, 2], F32, tag="parts")
                for h in range(2):
                    bh = h * HB
                    regA = rp.tile([128, RLEN], BF16, tag="regA")
                    nc.gpsimd.dma_start(regA[0:64, :], xp_d[img, :, bh:bh + RLEN])
                    nc.gpsimd.dma_start(regA[64:128, :], xp_d[img, :, bh + 1:bh + 1 + RLEN])
                    regB = rp.tile([128, RLEN], BF16, tag="regB")
                    nc.gpsimd.dma_start(regB[0:64, :], xp_d[img, :, bh + 2:bh + 2 + RLEN])
                    nc.gpsimd.dma_start(regB[64:128, :], xp_d[img, :, bh + 164:bh + 164 + RLEN])
                    regs.append((regA, regB))
                    rng = HB if h == 0 else RLEN
                    nc.vector.tensor_reduce(parts[:, h:h + 1], regA[0:64, 0:rng],
                                            axis=AX.X, op=ALU.add)

                # ---- router ----
                gbf = wp.tile([64, 1], BF16, tag="gbf")
                nc.vector.tensor_tensor(gbf[:, :], parts[:, 0:1], parts[:, 1:2], op=ALU.add)
                h_ps = pp.tile([16, 1], F32, tag="pr")
                nc.tensor.matmul(h_ps[:, :], fc1t_sb[:, :], gbf[:, :], start=True, stop=True)
                hbf = wp.tile([16, 1], BF16, tag="hbf")
                nc.scalar.activation(hbf[:, :], h_ps[:, :], AF.Relu)
                l_ps = pp.tile([4, 1], F32, tag="pr")
                nc.tensor.matmul(l_ps[:, :], fc2t_sb[:, :], hbf[:, :], start=True, stop=True)
                lsb = wp.tile([4, 1], F32, tag="lsb")
                nc.scalar.activation(lsb[:, :], l_ps[:, :], AF.Identity, bias=fc2b_sb[:, 0:1])
                lrow_ps = pp.tile([1, 4], F32, tag="pr")
                nc.tensor.transpose(lrow_ps[:, :], lsb[:, :], idc_sb[0:4, 0:4])
                # exp(l) = (1 + tanh(l/2)) / (1 - tanh(l/2)); logits are O(0.5)
                trow = wp.tile([1, 4], F32, tag="trow")
                nc.scalar.activation(trow[:, :], lrow_ps[:, :], AF.Tanh, scale=0.5)
                num = wp.tile([1, 4], F32, tag="num")
                nc.vector.tensor_scalar_add(num[:, :], trow[:, :], 1.0)
                den = wp.tile([1, 4], F32, tag="den")
                nc.vector.tensor_scalar(den[:, :], trow[:, :], -1.0, 1.0,
                                        op0=ALU.mult, op1=ALU.add)
                rec = wp.tile([1, 4], F32, tag="rec")
                nc.vector.reciprocal(rec[:, :], den[:, :])
                erow = wp.tile([1, 4], F32, tag="erow")
                nc.vector.tensor_tensor(erow[:, :], num[:, :], rec[:, :], op=ALU.mult)
                ssum = wp.tile([1, 1], F32, tag="ssum")
                nc.vector.tensor_reduce(ssum[:, :], erow[:, :], axis=AX.X, op=ALU.add)
                sinv = wp.tile([1, 1], F32, tag="sinv")
                nc.vector.reciprocal(sinv[:, :], ssum[:, :])
                wrow = wp.tile([1, 4], F32, tag="wrow")
                nc.vector.tensor_scalar_mul(wrow[:, :], erow[:, :], sinv[:, 0:1])
                rowA = wp.tile([1, 128], F32, tag="rowA")
                nc.vector.tensor_scalar_mul(rowA[:, 0:64], o64_sb[:, :], wrow[:, 0:1])
                nc.vector.tensor_scalar_mul(rowA[:, 64:128], o64_sb[:, :], wrow[:, 1:2])
                rowB = wp.tile([1, 128], F32, tag="rowB")
                nc.vector.tensor_scalar_mul(rowB[:, 0:64], o64_sb[:, :], wrow[:, 2:3])
                nc.vector.tensor_scalar_mul(rowB[:, 64:128], o64_sb[:, :], wrow[:, 3:4])
                wc_ps = pp.tile([128, 2], F32, tag="pr")
                nc.tensor.matmul(wc_ps[:, 0:1], rowA[:, :], idc_sb[0:1, 0:1],
                                 start=True, stop=True)
                nc.tensor.matmul(wc_ps[:, 1:2], rowB[:, :], idc_sb[0:1, 0:1],
                                 start=True, stop=True)
                wcol = wp.tile([128, 2], F32, tag="wcol")
                nc.scalar.copy(wcol[:, :], wc_ps[:, :])
                mixA = wp.tile([128, 128], BF16, tag="mixA")
                nc.scalar.activation(mixA[:, :], idc_sb[:, :], AF.Copy, scale=wcol[:, 0:1])
                mixB = wp.tile([128, 128], BF16, tag="mixB")
                nc.scalar.activation(mixB[:, :], idc_sb[:, :], AF.Copy, scale=wcol[:, 1:2])

                # ---- conv + silu + mix tiles ----
                for h in range(2):
                    regA, regB = regs[h]
                    for t in range(27):
                        N = NT if t < 26 else 324
                        n0 = t * NT
                        tsbs = []
                        for ep in range(2):
                            cps = pp.tile([128, N], F32, tag=f"pc{ep}", bufs=2)
                            base = ep * 512
                            nc.tensor.matmul(cps[:, :], wk_sb[:, base:base + 128],
                                             regA[:, n0:n0 + N], start=True, stop=False)
                            nc.tensor.matmul(cps[:, :], wk_sb[:, base + 128:base + 256],
                                             regA[:, n0 + 162:n0 + 162 + N],
                                             start=False, stop=False)
                            nc.tensor.matmul(cps[:, :], wk_sb[:, base + 256:base + 384],
                                             regA[:, n0 + 324:n0 + 324 + N],
                                             start=False, stop=False)
                            nc.tensor.matmul(cps[:, :], wk_sb[:, base + 384:base + 512],
                                             regB[:, n0:n0 + N], start=False, stop=False)
                            nc.tensor.matmul(cps[:, :], wks_sb[:, ep * 128:ep * 128 + 128],
                                             regB[:, n0 + 324:n0 + 324 + N],
                                             start=False, stop=True)
                            tsb = wp.tile([128, N], BF16, tag=f"t{ep}", bufs=12)
                            nc.scalar.activation(tsb[:, :], cps[:, :], AF.Silu,
                                                 bias=bnb_sb[:, ep:ep + 1])
                            tsbs.append(tsb)
                        po = pp.tile([128, N], F32, tag="po", bufs=3)
                        nc.tensor.matmul(po[:, :], mixA[:, :], tsbs[0][:, :],
                                         start=True, stop=False)
                        nc.tensor.matmul(po[:, :], mixB[:, :], tsbs[1][:, :],
                                         start=False, stop=True)
                        bounce = bp.tile([64, N], F32, tag="bounce")
                        nc.vector.tensor_copy(bounce[:, :], po[0:64, :])
                        r0 = h * 80 + t * 3
                        nrows = 3 if t < 26 else 2
                        bv = bounce[:, :].rearrange("p (r c) -> p r c", c=WP)[:, :, 0:W]
                        nc.sync.dma_start(out_d[img, :, r0:r0 + nrows, :], bv)

    nc.compile()
    return nc


def _prep_weights(fc1_w, fc2_w, fc2_b, conv_w, bn_gamma, bn_beta, bn_mean, bn_var):
    scale = bn_gamma / np.sqrt(bn_var + BN_EPS)            # [E, C]
    shift = bn_beta - bn_mean * scale                      # [E, C]
    ws = conv_w * scale[:, :, None, None, None]            # [E, Co, Ci, 3, 3]

    # paired-tap lhsT blocks: [K=128 (2 taps x 64 ci), M=128 (2 experts x 64 co)]
    groups = [((0, 0), (0, 1)), ((1, 0), (1, 1)), ((2, 0), (2, 1)), ((0, 2), (1, 2))]
    wk = np.zeros((128, 1024), np.float32)
    for ep in range(2):
        for g, (ta, tb) in enumerate(groups):
            blk = np.stack([ws[:, :, :, ta[0], ta[1]], ws[:, :, :, tb[0], tb[1]]])
            blk = blk[:, 2 * ep:2 * ep + 2]                # [j, le, Co, Ci]
            lhsT = blk.transpose(0, 3, 1, 2).reshape(128, 128)
            wk[:, (ep * 4 + g) * 128:(ep * 4 + g + 1) * 128] = lhsT
    wks = np.zeros((128, 256), np.float32)                 # rows 64-127 stay zero
    s22 = ws[:, :, :, 2, 2]                                # [E, Co, Ci]
    for ep in range(2):
        blk = s22[2 * ep:2 * ep + 2]                       # [le, Co, Ci]
        wks[0:64, ep * 128:(ep + 1) * 128] = blk.transpose(2, 0, 1).reshape(64, 128)

    return {
        "wk": wk.astype(BF16_NP),
        "wks": wks.astype(BF16_NP),
        "fc1t": (fc1_w.T / float(H * W)).astype(BF16_NP),  # [64, 16]
        "fc2t": fc2_w.T.astype(BF16_NP),                   # [16, 4]
        "fc2b": fc2_b.reshape(4, 1).astype(np.float32),
        "bnb": np.stack([np.concatenate([shift[0], shift[1]]),
                         np.concatenate([shift[2], shift[3]])], axis=1).astype(np.float32),
        "idc": np.concatenate([
            (np.arange(128)[:, None] % 64 == np.arange(64)[None, :]),
            np.zeros((128, 64), bool)], axis=1).astype(np.float32),
        "o64": np.ones((1, 64), np.float32),
    }


def kernel(x, fc1_w, fc2_w, fc2_b, conv_w, bn_gamma, bn_beta, bn_mean, bn_var):
    from concourse.bass_utils import run_bass_kernel_spmd

    x = np.asarray(x, np.float32)
    reps = int(os.environ.get("BASS_KERNEL_REPS", "1"))
    key = f"nc{reps}"
    if key not in _CACHE:
        _CACHE[key] = _build_program(reps)
    nc = _CACHE[key]

    wmap = _prep_weights(np.asarray(fc1_w, np.float32), np.asarray(fc2_w, np.float32),
                         np.asarray(fc2_b, np.float32), np.asarray(conv_w, np.float32),
                         np.asarray(bn_gamma, np.float32), np.asarray(bn_beta, np.float32),
                         np.asarray(bn_mean, np.float32), np.asarray(bn_var, np.float32))

    # host-side zero-pad + bf16 cast into the flat 162x162 (+margin) layout
    xp = np.zeros((B, C, LHOST), BF16_NP)
    xpad = xp[:, :, :LP].reshape(B, C, H + 2, WP)
    xpad[:, :, 1:H + 1, 1:W + 1] = x.astype(BF16_NP)

    in_maps = []
    for c in range(NCORES):
        m = dict(wmap)
        m["xp"] = xp[c * IMG_PER_CORE:(c + 1) * IMG_PER_CORE]
        in_maps.append(m)

    trace = bool(int(os.environ.get("BASS_KERNEL_TRACE", "0")))
    res = run_bass_kernel_spmd(nc, in_maps, list(range(NCORES)), trace=trace)
    _CACHE["last_results"] = res
    return np.concatenate([res.results[c]["out"] for c in range(NCORES)], axis=0)
